# revision 6
# baseline (speedup 1.0000x reference)
"""Trainium2 Bass kernel v2 for nn_CausalSelfAttention (BitLinear QKV/O + RoPE
+ causal attn).

Sharding: 2 heads x 2 batches per core (head-parallel), partial output
projection summed on host. Projections run as fp8 DoubleRow matmuls: q/k
straight off fp8(x); v off fp8(x) + fp8(x - fp8(x)) (residual) for ~1e-3
accuracy, transposed to [kv, d] on the PE. Flash-style attention in [kv, q]
score layout with unnormalized softmax; PV flipped to [q, d] via E-as-lhsT so
the denominator lands per-partition; causal mask added via an identity-lhsT
matmul of a constant mask block; 2-head-packed exp over a 2-bank PSUM tile.
"""
import sys

sys.path.insert(0, "/opt/trn_rl_repo")

import numpy as np

GROUP = 128
N_HEADS = 16
EPS = 1e-8
B, T, C = 2, 2048, 1024
HD = 64
N_CORES = 8
HPC = N_HEADS // N_CORES  # 2 heads per core
# DR weights are scaled by 2^6 into e4m3's normal range (raw ternary scales
# ~0.016 are subnormal in fp8); compensated in the exp scale / woC
W2K = 64.0
EXPS = 0.125 / (W2K * W2K)
MASKV = -1e7

try:
    import ml_dtypes

    BF16 = ml_dtypes.bfloat16
    F8E4 = ml_dtypes.float8_e4m3fn
except ImportError:  # pragma: no cover
    BF16 = np.float32
    F8E4 = np.float32


# ---------------------------------------------------------------- host prep
def _ternary_quantize(w):
    O, I = w.shape
    g = w.reshape(O, I // GROUP, GROUP).astype(np.float32)
    scale = np.maximum(np.mean(np.abs(g), axis=-1, keepdims=True), EPS).astype(
        np.float32
    )
    wn = g / scale
    q = np.where(wn > 0.5, 1.0, np.where(wn < -0.5, -1.0, 0.0)).astype(np.float32)
    return (q * scale).reshape(O, I).astype(np.float32)


def _fold_dr_x(xT):
    # [s][p][dr][j][u] = x[c = dr*256 + j*128 + p, t = s*512+u]
    return np.ascontiguousarray(
        xT.reshape(4, 2, 128, 8, 512).transpose(3, 2, 0, 1, 4)
    )


def _make_core_inputs(x, wq, wk, wv, wo, rope_cos, rope_sin):
    x = np.ascontiguousarray(x.astype(np.float32).reshape(B * T, C))
    wq_q = _ternary_quantize(wq)
    wk_q = _ternary_quantize(wk)
    wv_q = _ternary_quantize(wv)
    wo_q = _ternary_quantize(wo)

    xT = x.T  # [1024 c, 4096 t]
    x8f = xT.astype(F8E4)
    x8 = _fold_dr_x(x8f).astype(F8E4)
    r8 = _fold_dr_x((xT - x8f.astype(np.float32)).astype(F8E4))

    cosT = rope_cos.astype(np.float32).T  # [32, 2048]
    sinT = rope_sin.astype(np.float32).T
    cos_t = np.tile(cosT, (4, 1)).astype(BF16)
    sin_t = np.concatenate([-sinT, sinT, -sinT, sinT], axis=0).astype(BF16)
    r = np.arange(128)
    maskb = np.where(r[None, :] >= r[:, None], 0.0, MASKV).astype(BF16)
    identb = np.eye(128, dtype=np.float32).astype(BF16)

    maps = []
    for core in range(N_CORES):
        r0 = core * HPC * HD
        rows = slice(r0, r0 + HPC * HD)

        # w8[p][dr][j][d] = w[r0+d, c=dr*256+j*128+p]; weights pre-scaled by
        # W2K so the ~0.016-magnitude ternary scales are fp8-normal
        def w_fold(w_scaled):
            wsT = w_scaled[rows, :].T.reshape(4, 2, 128, 128)  # [dr, j, p, d]
            return np.ascontiguousarray(wsT.transpose(2, 0, 1, 3)).astype(F8E4)

        wv_s = (wv_q * W2K).astype(np.float32)
        wv8f = wv_s.astype(F8E4).astype(np.float32)
        maps.append(
            {
                "x8": x8,
                "r8": r8,
                "wq8": w_fold(wq_q * W2K),
                "wk8": w_fold(wk_q * W2K),
                "wv8": w_fold(wv_s),
                "wvr8": w_fold(wv_s - wv8f),
                "woC": np.ascontiguousarray(wo_q[:, rows].T / W2K).astype(BF16),
                "cos_t": cos_t,
                "sin_t": sin_t,
                "maskb": maskb,
                "identb": identb,
            }
        )
    return maps


# ---------------------------------------------------------------- BIR post-pass
def _split_excess_waits(nc, max_waits=1):
    import concourse.mybir as mybir

    for f in nc.m.functions:
        for bb in f.blocks:
            insts = bb.instructions
            i = 0
            while i < len(insts):
                ins = insts[i]
                si = ins.sync_info
                if si is not None and si.on_wait and len(si.on_wait) > max_waits:
                    waits = list(si.on_wait)
                    si.on_wait = waits[:max_waits]
                    rest = waits[max_waits:]
                    new_ops = []
                    for j in range(0, len(rest), max_waits):
                        new_ops.append(
                            mybir.InstNoOp(
                                name=nc.get_next_instruction_name(),
                                sync_info=mybir.SyncInfo(
                                    on_wait=rest[j : j + max_waits], on_update=[]
                                ),
                                bass_nofuse=True,
                                engine=ins.engine,
                            )
                        )
                    insts[i:i] = new_ops
                    i += len(new_ops)
                i += 1


# ---------------------------------------------------------------- device kernel
def _emit(nc, tc, d):
    import concourse.mybir as mybir
    from concourse.bass import AP, ds, ts

    f32 = mybir.dt.float32
    bf16 = mybir.dt.bfloat16
    f8 = mybir.dt.float8e4
    AF = mybir.ActivationFunctionType
    OP = mybir.AluOpType
    PM = mybir.MatmulPerfMode

    with nc.allow_low_precision(reason="bf16/fp8 compute, fp32 psum accum"), \
        tc.tile_pool(name="const", bufs=1) as cp0, \
        tc.tile_pool(name="persist", bufs=1) as pp, \
        tc.tile_pool(name="xtp", bufs=8) as xtp, \
        tc.tile_pool(name="vtp", bufs=6) as vtp, \
        tc.tile_pool(name="Ep", bufs=10) as Ep, \
        tc.tile_pool(name="rcp", bufs=6) as rcp, \
        tc.tile_pool(name="ynp", bufs=3) as ynp, \
        tc.tile_pool(name="obp", bufs=4) as obp, \
        tc.tile_pool(name="ropep", bufs=2) as ropep, \
        tc.tile_pool(name="sp2p", bufs=2, space="PSUM") as sp2p, \
        tc.tile_pool(name="ypp", bufs=1, space="PSUM") as ypp:

        # ---- constants (interleaved with x8 loads below for DMA priority)
        wq8 = cp0.tile([128, 4, 2, 128], f8)
        nc.sync.dma_start(wq8[:], d["wq8"])
        wk8 = cp0.tile([128, 4, 2, 128], f8)
        nc.sync.dma_start(wk8[:], d["wk8"])
        wv8 = cp0.tile([128, 4, 2, 128], f8)
        nc.sync.dma_start(wv8[:], d["wv8"])
        wvr8 = cp0.tile([128, 4, 2, 128], f8)
        nc.sync.dma_start(wvr8[:], d["wvr8"])

        qT = [pp.tile([128, 2048], bf16, name=f"qT{b}") for b in range(2)]
        kT = [pp.tile([128, 2048], bf16, name=f"kT{b}") for b in range(2)]
        v_sb = [pp.tile([128, 32 * 65], bf16, name=f"v_sb{b}") for b in range(2)]
        yT = [pp.tile([128, 2048], bf16, name=f"yT{b}") for b in range(2)]

        for b in range(2):
            ones_ap = AP(
                v_sb[b][:].tensor, v_sb[b][:].offset + 64, [[32 * 65, 128], [65, 32]]
            )
            nc.gpsimd.memset(ones_ap, 1.0)

        _DBG.update(qT=qT, kT=kT, v_sb=v_sb, yT=yT)

        x8s = [None] * 8
        r8s = [None] * 8

        def load(kind, s):
            t = xtp.tile([128, 4, 2, 512], f8, name=f"{kind}_t", tag=kind)
            nc.sync.dma_start(t[:], d[kind][s])
            (x8s if kind == "x8" else r8s)[s] = t

        def dr_chain(ps, w_t, x_t, start, stop):
            for dr in range(4):
                nc.tensor.matmul(
                    ps[:],
                    w_t[:, dr],
                    x_t[:, dr],
                    start=(start and dr == 0),
                    stop=(stop and dr == 3),
                    perf_mode=PM.DoubleRow,
                )

        def proj_qk(pool, s, w_t, destb, on_act=False):
            b, sc = s // 4, s % 4
            ps = pool.tile([128, 512], f32, name="psqk", tag="pr")
            dr_chain(ps, w_t, x8s[s], True, True)
            if on_act:
                nc.scalar.copy(destb[b][:, ts(sc, 512)], ps[:])
            else:
                nc.vector.tensor_copy(destb[b][:, ts(sc, 512)], ps[:])

        # v: two DR chains (x8 + residual) -> [d, t] psum -> bf16 sbuf -> PE
        # transpose -> v_sb [kv, d] blocks
        def v_a(pool, s, on_act=False):
            ps = pool.tile([128, 512], f32, name="psv", tag="pr")
            dr_chain(ps, wv8, x8s[s], True, False)
            dr_chain(ps, wv8, r8s[s], False, False)
            dr_chain(ps, wvr8, x8s[s], False, True)
            vt = vtp.tile([128, 512], bf16, name="vt")
            if on_act:
                nc.scalar.copy(vt[:], ps[:])
            else:
                nc.vector.tensor_copy(vt[:], ps[:])
            return vt

        def v_b(pool, s, vt):
            b = s // 4
            j0 = (s % 4) * 4
            ptr = pool.tile([128, 4, 128], bf16, name="vtr", tag="pr")
            for tb in range(4):
                nc.tensor.transpose(ptr[:, tb], vt[:, ts(tb, 128)], identb[:])
            src = AP(ptr.tensor, ptr.offset, [[512, 128], [128, 4], [64, 2], [1, 64]])
            dst = AP(
                v_sb[b][:].tensor,
                v_sb[b][:].offset + j0 * 65,
                [[32 * 65, 128], [65, 4], [16 * 65, 2], [1, 64]],
            )
            nc.vector.tensor_copy(dst, src)

        def rope_chunk(tile_, c0, w, sw_eng, mult_eng):
            sw = ropep.tile([128, 2048], bf16, name="sw", tag="sw")
            cc = ds(c0, w)
            sw_eng[0].dma_start(sw[0:32, cc], tile_[32:64, cc])
            sw_eng[1].dma_start(sw[32:64, cc], tile_[0:32, cc])
            sw_eng[0].dma_start(sw[64:96, cc], tile_[96:128, cc])
            sw_eng[1].dma_start(sw[96:128, cc], tile_[64:96, cc])
            tmp = ropep.tile([128, 2048], bf16, name="tmp", tag="tmp")
            nc.vector.tensor_tensor(tmp[:, cc], tile_[:, cc], cos_sb[:, cc], OP.mult)
            mult_eng.tensor_tensor(sw[:, cc], sw[:, cc], sin_sb[:, cc], OP.mult)
            nc.vector.tensor_tensor(tile_[:, cc], tmp[:, cc], sw[:, cc], OP.add)

        def unit(b, qi, cq, cpool, bg, st1=None, pv_delay=1):
            nj = 4 * qi + 4
            yph = [
                ypp.tile([128, 260], f32, name="ypA", tag="ypA"),
                ypp.tile([128, 260], f32, name="ypB", tag="ypB"),
            ]

            def pv(j, E):
                dlt = j * 128 - qi * 512
                dd = max(dlt // 128, 0)
                for h in range(2):
                    blk = h * 16 + j
                    for qb in range(dd, 4):
                        # start only on the bank's first matmul: start=True
                        # marks the whole 2KB zero region pending-zero, so a
                        # second start would wipe sibling qb accumulators
                        nc.tensor.matmul(
                            yph[h][:, ds(qb * 65, 65)],
                            E[:, ds(h * 512 + qb * 128, 128)],
                            v_sb[b][:, ds(blk * 65, 65)],
                            start=(j == 0 and qb == 0),
                            stop=(j == 4 * qi + qb),
                            skip_group_check=True,
                        )

            pend = []
            for j in range(nj):
                rem = nj - j
                take = max(1, (len(cq) + rem - 1) // rem) if cq else 0
                for _ in range(min(take, len(cq))):
                    cq.pop(0)(cpool)
                if st1 is not None and bg:
                    # even-paced: drain bg over ~36 stage-1 j-iterations so
                    # items rarely park the PE stream ahead of their input DMA
                    st1["jg"] += 1
                    quota = st1["bg0"] * st1["jg"] // 28
                    while bg and (st1["bg0"] - len(bg)) < quota:
                        bg.pop(0)(cpool)
                dlt = j * 128 - qi * 512
                q0 = max(dlt, 0)
                sp = sp2p.tile([128, 1024], f32, name="sp2")
                for h in range(2):
                    nc.tensor.matmul(
                        sp[:, ds(h * 512 + q0, 512 - q0)],
                        kT[b][64 * h : 64 * h + 64, ds(j * 128, 128)],
                        qT[b][64 * h : 64 * h + 64, ds(qi * 512 + q0, 512 - q0)],
                        start=True,
                        stop=(dlt < 0),
                        skip_group_check=True,
                    )
                    if dlt >= 0:
                        nc.tensor.matmul(
                            sp[:, ds(h * 512 + dlt, 128)],
                            identb[:],
                            maskb[:],
                            start=False,
                            stop=True,
                            skip_group_check=True,
                        )
                E = Ep.tile([128, 1024], bf16, name="E")
                e_in = AP(sp.tensor, sp.offset + q0, [[1024, 128], [512, 2], [1, 512 - q0]])
                e_out = AP(E.tensor, E.offset + q0, [[1024, 128], [512, 2], [1, 512 - q0]])
                nc.scalar.activation(e_out, e_in, AF.Exp, scale=EXPS)
                pend.append((j, E))
                if len(pend) > pv_delay:
                    pv(*pend.pop(0))
            while pend:
                pv(*pend.pop(0))
            # normalize: rc = 1/den, y_norm = yp * rc (stride-0 broadcast)
            y_norm = ynp.tile([128, 4, 128], bf16, name="y_norm")
            for h in range(2):
                rc = rcp.tile([128, 4], mybir.dt.float32, name="rc")
                den = AP(yph[h].tensor, yph[h].offset + 64, [[260, 128], [65, 4]])
                nc.vector.reciprocal(rc[:], den)
                src = AP(yph[h].tensor, yph[h].offset, [[260, 128], [65, 4], [1, 64]])
                rcb = AP(rc.tensor, rc.offset, [[4, 128], [1, 4], [0, 64]])
                dst = AP(
                    y_norm.tensor, y_norm.offset + h * 64, [[512, 128], [128, 4], [1, 64]]
                )
                nc.vector.tensor_tensor(dst, src, rcb, OP.mult)

            def fin(pool, y_norm=y_norm, b=b, qi=qi):
                ytr = pool.tile([128, 4, 128], bf16, name="ytr", tag="pr")
                for tb in range(4):
                    nc.tensor.transpose(ytr[:, tb], y_norm[:, tb], identb[:])
                ytb = yT[b][:]
                yt_dst = AP(ytb.tensor, ytb.offset + qi * 512, [[2048, 128], [1, 512]])
                yt_src = AP(ytr.tensor, ytr.offset, [[512, 128], [1, 512]])
                nc.vector.tensor_copy(yt_dst, yt_src)

            cq.append(fin)

        def make_c_items(b, qi, cq, last_unit=False):
            for tb in range(4):
                tcol = b * 16 + qi * 4 + tb

                def mk(pool, b=b, tci=qi * 4 + tb, tcol=tcol):
                    ob = obp.tile([128, 1024], bf16, name="ob")
                    for oc in range(2):
                        op = pool.tile([128, 512], f32, name="cps", tag="pr")
                        nc.tensor.matmul(
                            op[:],
                            yT[b][:, ts(tci, 128)],
                            woC[:, ts(oc, 512)],
                            start=True,
                            stop=True,
                        )
                        if last_unit:
                            # act is idle at the drain tail; parallel copies
                            if oc == 1:
                                nc.scalar.copy(ob[:, ts(oc, 512)], op[:])
                            else:
                                nc.vector.tensor_copy(ob[:, ts(oc, 512)], op[:])
                        else:
                            nc.vector.tensor_copy(ob[:, ts(oc, 512)], op[:])
                    nc.gpsimd.dma_start(d["outp"][ds(tcol * 128, 128), :], ob[:])

                cq.append(mk)

        # ================= stage 0: q/k projections b0 + rope(b0)
        with tc.tile_pool(name="prj", bufs=2, space="PSUM") as prj:
            load("x8", 0)
            load("x8", 1)
            cos_sb = cp0.tile([128, 2048], bf16)
            nc.sync.dma_start(cos_sb[:], d["cos_t"])
            sin_sb = cp0.tile([128, 2048], bf16)
            nc.sync.dma_start(sin_sb[:], d["sin_t"])
            maskb = cp0.tile([128, 128], bf16)
            nc.sync.dma_start(maskb[:], d["maskb"])
            identb = cp0.tile([128, 128], bf16)
            nc.sync.dma_start(identb[:], d["identb"])
            load("x8", 2)
            load("x8", 3)
            for s in range(4):
                proj_qk(prj, s, wq8, qT, on_act=True)
                proj_qk(prj, s, wk8, kT, on_act=True)
                if s % 2 == 1:
                    me = nc.vector if s == 1 else nc.gpsimd
                    for tile_ in (qT[0], kT[0]):
                        rope_chunk(tile_, (s - 1) * 512, 1024, (nc.sync, nc.scalar), me)
            for s in range(4):
                load("r8", s)
            woC = cp0.tile([128, 1024], bf16)
            nc.sync.dma_start(woC[:], d["woC"])
            for s in range(4, 8):
                load("x8", s)
            for s in range(4, 8):
                load("r8", s)

            # deferred work queued into B(b0) j-loops
            bg = []
            vts = {}

            def mk_va(s, on_act=False):
                def f(pool, s=s, on_act=on_act):
                    vts[s] = v_a(pool, s, on_act)

                return f

            def mk_vb(s):
                return lambda pool, s=s: v_b(pool, s, vts[s])

            for s in range(4):
                bg.append(mk_va(s, on_act=(s < 2)))
            for s in range(4):
                bg.append(mk_vb(s))
            for s in range(4, 8):
                bg.append(lambda pool, s=s: proj_qk(pool, s, wq8, qT))
                bg.append(lambda pool, s=s: proj_qk(pool, s, wk8, kT))

            def rope1q(pool):
                rope_chunk(qT[1], 0, 2048, (nc.gpsimd, nc.gpsimd), nc.gpsimd)

            def rope1k(pool):
                rope_chunk(kT[1], 0, 2048, (nc.gpsimd, nc.gpsimd), nc.gpsimd)

            bg.append(rope1q)
            bg.append(rope1k)
            for s in range(4, 8):
                bg.append(mk_va(s))
                bg.append(mk_vb(s))

            # ============= stage 1: attention b0, big units first
            cq = []
            st1 = {"jg": 0, "bg0": len(bg)}
            first = True
            for qi in (3, 2, 1, 0):
                unit(0, qi, cq, prj, bg, st1, pv_delay=8 if first else 4)
                first = False
                make_c_items(0, qi, cq)
            while bg:
                bg.pop(0)(prj)

            # ============= stage 2: attention b1 + deferred phase C
            first = True
            for qi in (3, 2):
                unit(1, qi, cq, prj, [], None, pv_delay=5 if first else 4)
                first = False
                make_c_items(1, qi, cq)
            unit(1, 1, cq, prj, [], None, pv_delay=4)
            make_c_items(1, 1, cq, last_unit=True)
            unit(1, 0, cq, prj, [], None, pv_delay=3)
            make_c_items(1, 0, cq, last_unit=True)
            while cq:
                cq.pop(0)(prj)


_NC_CACHE = {}
_DBG = {}


def _build(split_waits=True):
    if "nc" in _NC_CACHE:
        return _NC_CACHE["nc"]
    import concourse.bass as bass
    import concourse.mybir as mybir
    import concourse.tile as tile

    bf16 = mybir.dt.bfloat16
    f8 = mybir.dt.float8e4
    nc = bass.Bass("TRN2", target_bir_lowering=False, debug=False, num_devices=1)
    d = {
        "x8": nc.dram_tensor("x8", [8, 128, 4, 2, 512], f8, kind="ExternalInput").ap(),
        "r8": nc.dram_tensor("r8", [8, 128, 4, 2, 512], f8, kind="ExternalInput").ap(),
        "wq8": nc.dram_tensor("wq8", [128, 4, 2, 128], f8, kind="ExternalInput").ap(),
        "wk8": nc.dram_tensor("wk8", [128, 4, 2, 128], f8, kind="ExternalInput").ap(),
        "wv8": nc.dram_tensor("wv8", [128, 4, 2, 128], f8, kind="ExternalInput").ap(),
        "wvr8": nc.dram_tensor("wvr8", [128, 4, 2, 128], f8, kind="ExternalInput").ap(),
        "woC": nc.dram_tensor("woC", [128, 1024], bf16, kind="ExternalInput").ap(),
        "cos_t": nc.dram_tensor("cos_t", [128, 2048], bf16, kind="ExternalInput").ap(),
        "sin_t": nc.dram_tensor("sin_t", [128, 2048], bf16, kind="ExternalInput").ap(),
        "maskb": nc.dram_tensor("maskb", [128, 128], bf16, kind="ExternalInput").ap(),
        "identb": nc.dram_tensor("identb", [128, 128], bf16, kind="ExternalInput").ap(),
        "outp": nc.dram_tensor("outp", [4096, 1024], bf16, kind="ExternalOutput").ap(),
    }
    with tile.TileContext(nc) as tc:
        _emit(nc, tc, d)
    if split_waits:
        _split_excess_waits(nc)
    _NC_CACHE["nc"] = nc
    return nc


def kernel(x, wq, wk, wv, wo, rope_cos, rope_sin):
    from concourse import bass_utils

    x, wq, wk, wv, wo, rope_cos, rope_sin = (
        np.asarray(a, dtype=np.float32)
        for a in (x, wq, wk, wv, wo, rope_cos, rope_sin)
    )
    in_maps = _make_core_inputs(x, wq, wk, wv, wo, rope_cos, rope_sin)
    nc = _build()
    res = bass_utils.run_bass_kernel_spmd(nc, in_maps, core_ids=list(range(N_CORES)))
    total = np.zeros((B * T, C), np.float32)
    for i in range(N_CORES):
        total += np.asarray(res.results[i]["outp"], dtype=np.float32)
    return total.reshape(B, T, C).astype(np.float32)
